# revision 6
# baseline (speedup 1.0000x reference)
"""Dot-product attention (no softmax) on 8 TRN2 NeuronCores.

out[b,h] = (q[b,h] @ k[b,h].T) @ v[b,h]  for q,k,v [B,H,L,D] = [2,16,2048,64] f32.

Strategy: matmul associativity -> out = q @ (k.T @ v). KV = k.T@v is [64,64]
per head, so the problem collapses from O(L^2 D) to O(L D^2) flops and becomes
purely memory bound (48 MiB in / 16 MiB out per chip, 6+2 MiB per core).

Sharding: the 32 (b,h) attention instances are independent; each of the 8
cores handles 4 consecutive heads of the flattened (b*h) axis. No collectives.

v5 (trace-driven; baseline 41.5us -> v4 38.6us -> this):
- fp32 matmuls cost 4 cycles/row on the PE vs 1 for bf16 (fp32 transposes 2),
  so k,v are cast f32->bf16 on the DVE (SBUF tensor_copy runs 2 elem/cycle/
  lane even from f32; ~1.2us per head). q is never explicitly cast: the PE
  transpose reads f32 and the PSUM->SBUF copy of the slab writes bf16.
- PSUM->SBUF copies are batched 4-wide (one 2 KiB bank) to amortize ~150ns
  per-instruction overhead; they run on the scalar engine, except the last
  two heads where the two half-head copies run on ACT and DVE in parallel
  (they are on the store-tail critical path).
- Load order q0,kv0,qkv1,kv2,kv3,q2,q3: the kv planes of the last two heads
  arrive mid-stream so their cast->KV->fixup chains finish under the load
  stream; only the cheap q-side chain (transpose -> qt copy -> out matmul ->
  copy -> store) remains after the last byte lands. In v4 the whole
  cast+KV+fixup chain sat after the last load and ran on a cold (1.2 GHz,
  HAM-throttled) PE.
- Stores interleave with loads on the sync HWDGE queue (store dma_starts are
  emitted after all load dma_starts, so their semaphore waits cannot delay a
  load).

Per-core layout trick: a head's [2048, 64] tensor is viewed as [128, 16, 64]
(partition p holds rows 16p..16p+15, 4 KiB contiguous DRAM per partition, so
every DMA is fully coalesced). The KV reduction over L is order-independent,
and the same interleaved row mapping flows through transpose -> matmul ->
store unchanged.
"""

import sys

if "/opt/trn_rl_repo" not in sys.path:
    sys.path.insert(0, "/opt/trn_rl_repo")

from contextlib import ExitStack

import numpy as np

import concourse.bass as bass
import concourse.tile as tile
from concourse import bacc, mybir
from concourse.bass_utils import run_bass_kernel_spmd

B, H, L, D = 2, 16, 2048, 64
N_CORES = 8
HPC = (B * H) // N_CORES  # heads per core = 4
P = 128
J = L // P  # 16 row-slots per partition
F32 = mybir.dt.float32
BF16 = mybir.dt.bfloat16


def _body(ctx: ExitStack, tc: tile.TileContext, o_d, qkv_d):
    nc = tc.nc

    const_pool = ctx.enter_context(tc.tile_pool(name="const", bufs=1))
    in_pool = ctx.enter_context(tc.tile_pool(name="in", bufs=4))
    kvb_pool = ctx.enter_context(tc.tile_pool(name="kvb", bufs=4))
    qt_pool = ctx.enter_context(tc.tile_pool(name="qt", bufs=8))
    kv_pool = ctx.enter_context(tc.tile_pool(name="kv", bufs=4))
    out_pool = ctx.enter_context(tc.tile_pool(name="out", bufs=4))
    psum_kv = ctx.enter_context(tc.tile_pool(name="psum_kv", bufs=1, space="PSUM"))
    psum_s = ctx.enter_context(tc.tile_pool(name="psum_s", bufs=1, space="PSUM"))
    psum_t = ctx.enter_context(tc.tile_pool(name="psum_t", bufs=2, space="PSUM"))
    psum_o = ctx.enter_context(tc.tile_pool(name="psum_o", bufs=2, space="PSUM"))
    psum_w = ctx.enter_context(tc.tile_pool(name="psum_w", bufs=1, space="PSUM"))

    # Per-head fused qkv tile: [128, 3(q/k/v), 16, 64] f32 (12 KiB/partition).
    # kv_sbs holds the bf16 casts of the k,v planes.
    qkv_sbs = [
        in_pool.tile([P, 3, J, D], F32, tag="qkv", name=f"qkv{h}") for h in range(HPC)
    ]
    kv_sbs = [
        kvb_pool.tile([P, 2, J, D], BF16, tag="kvb", name=f"kvb{h}")
        for h in range(HPC)
    ]

    def qkv_view(h):
        # [3, L, D] f32 in DRAM -> [p, t, j, d]; per partition chunks of
        # 4 KiB (q/k/v planes), fully coalesced descriptors.
        return qkv_d[h].rearrange("t (p j) d -> p t j d", p=P)

    # All loads issued up front on the sync queue (HWDGE), in dependency-
    # urgency order: kv planes of the tail heads before their q planes, the
    # q planes of the last two heads last (their post-load chain is cheap).
    nc.sync.dma_start(qkv_sbs[0][:, 0], qkv_view(0)[:, 0])  # q0
    nc.sync.dma_start(qkv_sbs[0][:, 1:3], qkv_view(0)[:, 1:3])  # kv0
    nc.sync.dma_start(qkv_sbs[1][:], qkv_view(1))  # qkv1
    nc.sync.dma_start(qkv_sbs[2][:, 1:3], qkv_view(2)[:, 1:3])  # kv2
    nc.sync.dma_start(qkv_sbs[3][:, 1:3], qkv_view(3)[:, 1:3])  # kv3
    nc.sync.dma_start(qkv_sbs[2][:, 0], qkv_view(2)[:, 0])  # q2
    nc.sync.dma_start(qkv_sbs[3][:, 0], qkv_view(3)[:, 0])  # q3

    # HAM warm-up: ~3.4us of dense bf16 matmuls while the first DMAs are in
    # flight, so the PE clock un-throttles (4/8 -> 8/8) before the real
    # transposes/matmuls start. Results are never read.
    warm_in = const_pool.tile([P, 4 * P], BF16)
    nc.vector.memset(warm_in[:], 0.0)
    warm_ps = psum_w.tile([P, 4 * P], F32)
    for _ in range(8):
        nc.tensor.matmul(
            warm_ps[:], warm_in[:, 0:P], warm_in[:], start=True, stop=True
        )

    # Identity (f32, matching q's dtype) for PE transposes.
    ident = const_pool.tile([P, P], F32)
    nc.gpsimd.memset(ident[:], 0.0)
    nc.gpsimd.affine_select(
        out=ident[:],
        in_=ident[:],
        compare_op=mybir.AluOpType.not_equal,
        fill=1.0,
        base=0,
        pattern=[[-1, P]],
        channel_multiplier=1,
    )

    # ones_dbl[p, m] = 1 iff p == m (mod 64): one matmul against it both sums
    # the two column-tiled KV halves and replicates the result to partitions
    # 64..127 (the odd-slot block of KV2).
    ones_dbl = const_pool.tile([P, P], BF16)
    nc.gpsimd.memset(ones_dbl[:], 0.0)
    for off in (-64, 0, 64):
        nc.gpsimd.affine_select(
            out=ones_dbl[:],
            in_=ones_dbl[:],
            compare_op=mybir.AluOpType.not_equal,
            fill=1.0,
            base=-off,
            pattern=[[-1, P]],
            channel_multiplier=1,
        )

    qts_all = [None] * HPC
    kv2s = [None] * HPC

    def emit_T(h):
        """Transpose q_h: 4 slab-pairs per PSUM bank (f32), then one batched
        copy per bank (f32 PSUM -> bf16 SBUF). For the tail heads the two
        bank copies run on ACT and DVE in parallel."""
        q_sb = qkv_sbs[h][:, 0]
        qts = []
        for g in range(2):
            qt_ps = psum_t.tile([P, 4, P], F32, tag="qt_ps")
            for i in range(4):
                jp = 4 * g + i
                nc.tensor.matmul(
                    qt_ps[:, i],
                    q_sb[:, 2 * jp : 2 * jp + 2],
                    ident[:],
                    is_transpose=True,
                    start=True,
                    stop=True,
                    skip_group_check=True,
                )
            qt_sb = qt_pool.tile([P, 4, P], BF16, tag="qt", name=f"qt{h}_{g}")
            if h >= 2 and g == 1:
                nc.vector.tensor_copy(qt_sb[:], qt_ps[:])
            else:
                nc.scalar.activation(
                    qt_sb[:], qt_ps[:], mybir.ActivationFunctionType.Identity
                )
            qts.append(qt_sb)
        qts_all[h] = qts

    def emit_kv_chain(h):
        """Cast k/v_h -> bf16, KV accumulation, KV2 = blockdiag(KV, KV)."""
        k_sb = kv_sbs[h][:, 0]
        v_sb = kv_sbs[h][:, 1]

        # k,v f32 -> bf16: one DVE copy (2 elem/cycle/lane, ~1.2us).
        nc.vector.tensor_copy(kv_sbs[h][:], qkv_sbs[h][:, 1:3])

        # KV = k.T @ v, column-tiled: even j-slots accumulate into PE columns
        # 0..63 (psum partitions 0..63), odd slots into columns 64..127, so
        # the two matmuls of a pair run concurrently in the array.
        kv_ps = psum_kv.tile([P, D], F32)
        for jp in range(J // 2):
            nc.tensor.matmul(
                kv_ps[0:D],
                k_sb[:, 2 * jp],
                v_sb[:, 2 * jp],
                start=(jp == 0),
                stop=(jp == J // 2 - 1),
                tile_position=(0, 0),
                skip_group_check=True,
            )
            nc.tensor.matmul(
                kv_ps[D : 2 * D],
                k_sb[:, 2 * jp + 1],
                v_sb[:, 2 * jp + 1],
                start=(jp == 0),
                stop=(jp == J // 2 - 1),
                tile_position=(0, D),
                skip_group_check=True,
            )
        kv_raw = kv_pool.tile([P, D], BF16, tag="kv_raw", name=f"kvr{h}")
        nc.vector.tensor_copy(kv_raw[:], kv_ps[:])
        kv_st_ps = psum_s.tile([P, D], F32, tag="kv_st", name=f"kvs{h}")
        nc.tensor.matmul(kv_st_ps[:], ones_dbl[:], kv_raw[:], start=True, stop=True)
        # KV2 = blockdiag(KV, KV): one [128,128] matmul against it computes two
        # output slots at once (lhsT = a transposed q slab pair).
        kv2 = kv_pool.tile([P, P], BF16, tag="kv2", name=f"kv2_{h}")
        nc.gpsimd.memset(kv2[:], 0.0)
        nc.vector.tensor_copy(kv2[0:D, 0:D], kv_st_ps[0:D])
        nc.vector.tensor_copy(kv2[D : 2 * D, D : 2 * D], kv_st_ps[D : 2 * D])
        kv2s[h] = kv2

    def emit_O(h):
        out_sb = out_pool.tile([P, J, D], F32, tag="o", name=f"o{h}")
        ov = o_d[h].rearrange("(p j) d -> p j d", p=P)
        kv2 = kv2s[h]
        for g in range(2):
            o_ps = psum_o.tile([P, 8, D], F32, tag="o_ps")
            for i in range(4):
                nc.tensor.matmul(
                    o_ps[:, 2 * i : 2 * i + 2],
                    qts_all[h][g][:, i],
                    kv2[:],
                    start=True,
                    stop=True,
                    skip_group_check=True,
                )
            half = slice(8 * g, 8 * g + 8)
            if h >= 2 and g == 1:
                nc.vector.tensor_copy(out_sb[:, half], o_ps[:])
            else:
                nc.scalar.activation(
                    out_sb[:, half], o_ps[:], mybir.ActivationFunctionType.Identity
                )
            nc.sync.dma_start(ov[:, half], out_sb[:, half])

    # Emission order ~= per-engine stream order. Heads 0/1 run their full
    # chains as loads land; heads 2/3 run their kv chains mid-stream and only
    # the cheap q-side chain after their (last-loaded) q planes arrive.
    emit_T(0)
    emit_kv_chain(0)
    emit_O(0)
    emit_T(1)
    emit_kv_chain(1)
    emit_O(1)
    emit_kv_chain(2)
    emit_kv_chain(3)
    emit_T(2)
    emit_O(2)
    emit_T(3)
    emit_O(3)


def build():
    nc = bacc.Bacc("TRN2", target_bir_lowering=False, debug=False)
    qkv_d = nc.dram_tensor("qkv", [HPC, 3, L, D], F32, kind="ExternalInput").ap()
    o_d = nc.dram_tensor("out", [HPC, L, D], F32, kind="ExternalOutput").ap()
    with tile.TileContext(nc) as tc, ExitStack() as ctx:
        _body(ctx, tc, o_d, qkv_d)
    nc.compile()
    return nc


_NC = None


def _get_nc():
    global _NC
    if _NC is None:
        _NC = build()
    return _NC


def make_in_maps(q, k, v):
    qf = np.asarray(q, dtype=np.float32).reshape(B * H, L, D)
    kf = np.asarray(k, dtype=np.float32).reshape(B * H, L, D)
    vf = np.asarray(v, dtype=np.float32).reshape(B * H, L, D)
    # [B*H, 3, L, D]: per head q/k/v adjacent so one DMA loads a whole head.
    qkv = np.stack([qf, kf, vf], axis=1)
    return [
        {"qkv": np.ascontiguousarray(qkv[c * HPC : (c + 1) * HPC])}
        for c in range(N_CORES)
    ]


def run_sharded(q, k, v, **spmd_kwargs):
    """Run on all 8 cores; returns (full_output, BassKernelResults)."""
    nc = _get_nc()
    res = run_bass_kernel_spmd(
        nc, make_in_maps(q, k, v), core_ids=list(range(N_CORES)), **spmd_kwargs
    )
    shards = [np.asarray(res.results[c]["out"]) for c in range(N_CORES)]
    out = np.concatenate(shards, axis=0).reshape(B, H, L, D).astype(np.float32)
    return out, res


def kernel(q, k, v):
    out, _ = run_sharded(q, k, v)
    return out


# revision 8
# speedup vs baseline: 1.0550x; 1.0550x over previous
"""Dot-product attention (no softmax) on 8 TRN2 NeuronCores.

out[b,h] = (q[b,h] @ k[b,h].T) @ v[b,h]  for q,k,v [B,H,L,D] = [2,16,2048,64] f32.

Strategy: matmul associativity -> out = q @ (k.T @ v). KV = k.T@v is [64,64]
per head, so the problem collapses from O(L^2 D) to O(L D^2) flops and becomes
purely memory bound (6 MiB in / 2 MiB out per core; ~20us at the measured
~420 GB/s per-core DMA rate).

Sharding: the 32 (b,h) attention instances are independent; each of the 8
cores handles 4 consecutive heads of the flattened (b*h) axis. No collectives.

v6 (trace-driven; baseline 41.5us -> v4 38.6us -> this):
- All compute in bf16 (fp32 matmuls cost 4 cycles/row vs 1, fp32 transposes
  2): k,v cast on the DVE (2 elem/cycle/lane from SBUF); q is consumed f32
  by the PE transpose whose PSUM->SBUF copy writes bf16.
- PSUM->SBUF copies batched 4-wide per 2 KiB bank (amortizes ~150ns/instr),
  on the scalar engine; tail copies split ACT/DVE so they run in parallel.
- Per head the transposes are emitted BEFORE the KV matmuls: the transposes
  only need the (f32) load while KV needs the DVE cast, so the PE works
  during the cast instead of idling.
- Keep-warm matmul bundles fill the two multi-us PE idle windows of this
  schedule. Without them the HAM clock gate re-throttles the PE to 1.2 GHz
  (one idle 4096-cycle window) and the whole tail runs at half clock
  (v4: 14.7us throttle-active, tail transposes measured 420ns vs 214).
- The last head is loaded kv3 -> q3[slots 0:12] -> q3[slots 12:16] and its
  output tail is chunked (4+2+2 slab pairs) with parallel ACT/DVE copies, so
  after the last byte lands only a ~1.5us transpose->matmul->copy->store
  chain remains; the cast+KV+fixup chain of head 3 hides under the q3
  stream.

Per-core layout trick: a head's [2048, 64] tensor is viewed as [128, 16, 64]
(partition p holds rows 16p..16p+15, 4 KiB contiguous DRAM per partition, so
every DMA is fully coalesced). The KV reduction over L is order-independent,
and the same interleaved row mapping flows through transpose -> matmul ->
store unchanged.
"""

import sys

if "/opt/trn_rl_repo" not in sys.path:
    sys.path.insert(0, "/opt/trn_rl_repo")

from contextlib import ExitStack

import numpy as np

import concourse.bass as bass
import concourse.tile as tile
from concourse import bacc, mybir
from concourse.bass_utils import run_bass_kernel_spmd

B, H, L, D = 2, 16, 2048, 64
N_CORES = 8
HPC = (B * H) // N_CORES  # heads per core = 4
P = 128
J = L // P  # 16 row-slots per partition
F32 = mybir.dt.float32
BF16 = mybir.dt.bfloat16


def _body(ctx: ExitStack, tc: tile.TileContext, o_d, qkv_d):
    nc = tc.nc

    const_pool = ctx.enter_context(tc.tile_pool(name="const", bufs=1))
    in_pool = ctx.enter_context(tc.tile_pool(name="in", bufs=4))
    kvb_pool = ctx.enter_context(tc.tile_pool(name="kvb", bufs=4))
    qt_pool = ctx.enter_context(tc.tile_pool(name="qt", bufs=8))
    kv_pool = ctx.enter_context(tc.tile_pool(name="kv", bufs=4))
    out_pool = ctx.enter_context(tc.tile_pool(name="out", bufs=4))
    psum_kv = ctx.enter_context(tc.tile_pool(name="psum_kv", bufs=1, space="PSUM"))
    psum_s = ctx.enter_context(tc.tile_pool(name="psum_s", bufs=1, space="PSUM"))
    psum_t = ctx.enter_context(tc.tile_pool(name="psum_t", bufs=2, space="PSUM"))
    psum_o = ctx.enter_context(tc.tile_pool(name="psum_o", bufs=2, space="PSUM"))
    psum_w = ctx.enter_context(tc.tile_pool(name="psum_w", bufs=1, space="PSUM"))

    qkv_sbs = [
        in_pool.tile([P, 3, J, D], F32, tag="qkv", name=f"qkv{h}") for h in range(HPC)
    ]
    kv_sbs = [
        kvb_pool.tile([P, 2, J, D], BF16, tag="kvb", name=f"kvb{h}")
        for h in range(HPC)
    ]

    def qkv_view(h):
        # [3, L, D] f32 in DRAM -> [p, t, j, d]; per partition chunks of
        # 4 KiB (q/k/v planes), fully coalesced descriptors.
        return qkv_d[h].rearrange("t (p j) d -> p t j d", p=P)

    # All loads issued up front on the sync queue (HWDGE). Store dma_starts
    # are emitted later, so their semaphore waits cannot delay a load.
    hl = HPC - 1
    for h in range(HPC - 1):
        nc.sync.dma_start(qkv_sbs[h][:], qkv_view(h))
    nc.sync.dma_start(qkv_sbs[hl][:, 1:3], qkv_view(hl)[:, 1:3])  # kv3
    nc.sync.dma_start(qkv_sbs[hl][:, 0, 0:12], qkv_view(hl)[:, 0, 0:12])  # q3a
    nc.sync.dma_start(qkv_sbs[hl][:, 0, 12:J], qkv_view(hl)[:, 0, 12:J])  # q3b

    # HAM warm-up: dense bf16 matmuls while the first load is in flight so
    # the PE clock un-throttles (4/8 -> 8/8). Results are never read.
    warm_in = const_pool.tile([P, 4 * P], BF16)
    nc.vector.memset(warm_in[:], 0.0)
    warm_ps = psum_w.tile([P, 4 * P], F32)

    def warm_bundle(n):
        for _ in range(n):
            nc.tensor.matmul(
                warm_ps[:], warm_in[:, 0:P], warm_in[:], start=True, stop=True
            )

    warm_bundle(8)

    # Identity (f32, matching q's dtype) for PE transposes.
    ident = const_pool.tile([P, P], F32)
    nc.gpsimd.memset(ident[:], 0.0)
    nc.gpsimd.affine_select(
        out=ident[:],
        in_=ident[:],
        compare_op=mybir.AluOpType.not_equal,
        fill=1.0,
        base=0,
        pattern=[[-1, P]],
        channel_multiplier=1,
    )

    # ones_dbl[p, m] = 1 iff p == m (mod 64): one matmul against it both sums
    # the two column-tiled KV halves and replicates the result to partitions
    # 64..127 (the odd-slot block of KV2).
    ones_dbl = const_pool.tile([P, P], BF16)
    nc.gpsimd.memset(ones_dbl[:], 0.0)
    for off in (-64, 0, 64):
        nc.gpsimd.affine_select(
            out=ones_dbl[:],
            in_=ones_dbl[:],
            compare_op=mybir.AluOpType.not_equal,
            fill=1.0,
            base=-off,
            pattern=[[-1, P]],
            channel_multiplier=1,
        )

    qts_all = [[None, None] for _ in range(HPC)]
    kv2s = [None] * HPC

    def emit_T_group(h, g, npairs=4, dve=False):
        """Transpose q_h slab-pairs 4g..4g+npairs-1 into one PSUM bank, then
        one batched copy (f32 PSUM -> bf16 SBUF) on ACT (or DVE)."""
        q_sb = qkv_sbs[h][:, 0]
        qt_ps = psum_t.tile([P, 4, P], F32, tag="qt_ps")
        for i in range(npairs):
            jp = 4 * g + i
            nc.tensor.matmul(
                qt_ps[:, i],
                q_sb[:, 2 * jp : 2 * jp + 2],
                ident[:],
                is_transpose=True,
                start=True,
                stop=True,
                skip_group_check=True,
            )
        qt_sb = qt_pool.tile([P, 4, P], BF16, tag="qt", name=f"qt{h}_{g}")
        if dve:
            nc.vector.tensor_copy(qt_sb[:, 0:npairs], qt_ps[:, 0:npairs])
        else:
            nc.scalar.activation(
                qt_sb[:, 0:npairs],
                qt_ps[:, 0:npairs],
                mybir.ActivationFunctionType.Identity,
            )
        qts_all[h][g] = qt_sb

    def emit_kv_chain(h):
        """Cast k/v_h -> bf16, KV accumulation, KV2 = blockdiag(KV, KV)."""
        k_sb = kv_sbs[h][:, 0]
        v_sb = kv_sbs[h][:, 1]

        nc.vector.tensor_copy(kv_sbs[h][:], qkv_sbs[h][:, 1:3])

        # KV = k.T @ v, column-tiled: even j-slots accumulate into PE columns
        # 0..63, odd slots into 64..127, so pair matmuls run concurrently.
        kv_ps = psum_kv.tile([P, D], F32)
        for jp in range(J // 2):
            nc.tensor.matmul(
                kv_ps[0:D],
                k_sb[:, 2 * jp],
                v_sb[:, 2 * jp],
                start=(jp == 0),
                stop=(jp == J // 2 - 1),
                tile_position=(0, 0),
                skip_group_check=True,
            )
            nc.tensor.matmul(
                kv_ps[D : 2 * D],
                k_sb[:, 2 * jp + 1],
                v_sb[:, 2 * jp + 1],
                start=(jp == 0),
                stop=(jp == J // 2 - 1),
                tile_position=(0, D),
                skip_group_check=True,
            )
        kv_raw = kv_pool.tile([P, D], BF16, tag="kv_raw", name=f"kvr{h}")
        nc.vector.tensor_copy(kv_raw[:], kv_ps[:])
        kv_st_ps = psum_s.tile([P, D], F32, tag="kv_st", name=f"kvs{h}")
        nc.tensor.matmul(kv_st_ps[:], ones_dbl[:], kv_raw[:], start=True, stop=True)
        kv2 = kv_pool.tile([P, P], BF16, tag="kv2", name=f"kv2_{h}")
        nc.gpsimd.memset(kv2[:], 0.0)
        nc.vector.tensor_copy(kv2[0:D, 0:D], kv_st_ps[0:D])
        nc.vector.tensor_copy(kv2[D : 2 * D, D : 2 * D], kv_st_ps[D : 2 * D])
        kv2s[h] = kv2

    out_sbs = [
        out_pool.tile([P, J, D], F32, tag="o", name=f"o{h}") for h in range(HPC)
    ]

    def emit_O_group(h, g, goff=0, npairs=4, dve=False):
        """Out matmuls for slab-pairs (4g+goff)..(4g+goff+npairs-1), batched
        copy on ACT (or DVE), then the store."""
        out_sb = out_sbs[h]
        ov = o_d[h].rearrange("(p j) d -> p j d", p=P)
        o_ps = psum_o.tile([P, 8, D], F32, tag="o_ps")
        for i in range(npairs):
            nc.tensor.matmul(
                o_ps[:, 2 * i : 2 * i + 2],
                qts_all[h][g][:, goff + i],
                kv2s[h][:],
                start=True,
                stop=True,
                skip_group_check=True,
            )
        lo = 8 * g + 2 * goff
        half = slice(lo, lo + 2 * npairs)
        if dve:
            nc.vector.tensor_copy(out_sb[:, half], o_ps[:, 0 : 2 * npairs])
        else:
            nc.scalar.activation(
                out_sb[:, half],
                o_ps[:, 0 : 2 * npairs],
                mybir.ActivationFunctionType.Identity,
            )
        nc.sync.dma_start(ov[:, half], out_sb[:, half])

    def emit_head(h):
        emit_T_group(h, 0)
        emit_T_group(h, 1)
        emit_kv_chain(h)

    def emit_O(h):
        emit_O_group(h, 0)
        emit_O_group(h, 1)

    # Heads 0-2: full chains as their fused loads land; O phases interleaved
    # so stores overlap later loads. Keep-warm bundles fill the two PE idle
    # windows (post-head-0 and pre-tail) so the tail runs at 2.4 GHz.
    emit_head(0)
    warm_bundle(12)
    emit_head(1)
    emit_O(0)
    emit_head(2)
    emit_O(1)
    emit_O(2)
    warm_bundle(8)
    # Head 3: kv chain hides under the q3 stream; output tail chunked 4+2+2
    # with ACT/DVE copies alternating so consecutive chunks overlap.
    emit_kv_chain(3)
    emit_T_group(3, 0, npairs=4, dve=False)  # jp 0-3 (q3a)
    q3_sb = qkv_sbs[3][:, 0]
    qt3b = qt_pool.tile([P, 4, P], BF16, tag="qt", name="qt3_1")
    qts_all[3][1] = qt3b
    qt_ps_b = psum_t.tile([P, 4, P], F32, tag="qt_ps")
    for i, jp in enumerate((4, 5)):  # still q3a
        nc.tensor.matmul(
            qt_ps_b[:, i],
            q3_sb[:, 2 * jp : 2 * jp + 2],
            ident[:],
            is_transpose=True,
            start=True,
            stop=True,
            skip_group_check=True,
        )
    nc.vector.tensor_copy(qt3b[:, 0:2], qt_ps_b[:, 0:2])
    emit_O_group(3, 0, goff=0, npairs=4, dve=False)  # slots 0:8
    emit_O_group(3, 1, goff=0, npairs=2, dve=True)  # slots 8:12
    qt_ps_c = psum_t.tile([P, 4, P], F32, tag="qt_ps")
    for i, jp in enumerate((6, 7)):  # q3b
        nc.tensor.matmul(
            qt_ps_c[:, i],
            q3_sb[:, 2 * jp : 2 * jp + 2],
            ident[:],
            is_transpose=True,
            start=True,
            stop=True,
            skip_group_check=True,
        )
    nc.vector.tensor_copy(qt3b[:, 2:4], qt_ps_c[:, 0:2])
    emit_O_group(3, 1, goff=2, npairs=2, dve=False)  # slots 12:16


def build():
    nc = bacc.Bacc("TRN2", target_bir_lowering=False, debug=False)
    qkv_d = nc.dram_tensor("qkv", [HPC, 3, L, D], F32, kind="ExternalInput").ap()
    o_d = nc.dram_tensor("out", [HPC, L, D], F32, kind="ExternalOutput").ap()
    with tile.TileContext(nc) as tc, ExitStack() as ctx:
        _body(ctx, tc, o_d, qkv_d)
    nc.compile()
    return nc


_NC = None


def _get_nc():
    global _NC
    if _NC is None:
        _NC = build()
    return _NC


def make_in_maps(q, k, v):
    qf = np.asarray(q, dtype=np.float32).reshape(B * H, L, D)
    kf = np.asarray(k, dtype=np.float32).reshape(B * H, L, D)
    vf = np.asarray(v, dtype=np.float32).reshape(B * H, L, D)
    # [B*H, 3, L, D]: per head q/k/v adjacent so one DMA loads a whole head.
    qkv = np.stack([qf, kf, vf], axis=1)
    return [
        {"qkv": np.ascontiguousarray(qkv[c * HPC : (c + 1) * HPC])}
        for c in range(N_CORES)
    ]


def run_sharded(q, k, v, **spmd_kwargs):
    """Run on all 8 cores; returns (full_output, BassKernelResults)."""
    nc = _get_nc()
    res = run_bass_kernel_spmd(
        nc, make_in_maps(q, k, v), core_ids=list(range(N_CORES)), **spmd_kwargs
    )
    shards = [np.asarray(res.results[c]["out"]) for c in range(N_CORES)]
    out = np.concatenate(shards, axis=0).reshape(B, H, L, D).astype(np.float32)
    return out, res


def kernel(q, k, v):
    out, _ = run_sharded(q, k, v)
    return out


# revision 9
# speedup vs baseline: 1.0628x; 1.0074x over previous
"""Dot-product attention (no softmax) on 8 TRN2 NeuronCores.

out[b,h] = (q[b,h] @ k[b,h].T) @ v[b,h]  for q,k,v [B,H,L,D] = [2,16,2048,64] f32.

Strategy: matmul associativity -> out = q @ (k.T @ v). KV = k.T@v is [64,64]
per head, so the problem collapses from O(L^2 D) to O(L D^2) flops and becomes
purely memory bound (6 MiB in / 2 MiB out per core; ~20us at the measured
~420 GB/s per-core DMA rate).

Sharding: the 32 (b,h) attention instances are independent; each of the 8
cores handles 4 consecutive heads of the flattened (b*h) axis. No collectives.

v7 (trace-driven; baseline 41.5us -> v4 38.6us -> this). Measured constraints
this schedule is built around:
- A DMA's completion semaphore fires ~1.5-2.5us after its last byte (HBM
  receipt under load), so every dependency boundary on a load costs that
  latency on top of stream position.
- fp32 matmuls cost 4 cycles/row vs 1 for bf16 (fp32 transposes 2cyc via a
  LOW/HIGH double pass), so k,v are cast f32->bf16 on the DVE (2 elem/cycle/
  lane from SBUF); q is consumed f32 by the PE transpose whose PSUM->SBUF
  copy writes bf16 (no separate q cast).
- The HAM clock gate halves the PE clock after any ~3.4us idle window; the
  warm-up is sized to bridge to the first load's semaphore (~24 matmuls) and
  one mid bundle covers the one unavoidable data gap.

Schedule: loads [qkv0, qkv1, kv2, kv3, q2, q3a, q3b]. Heads 0/1 run full
chains as their fused loads land; heads 2/3's cast->KV->KV2 chains complete
under the load stream (kv planes arrive mid-stream), so after the last q
bytes only a short transpose -> out-matmul -> copy -> store chain remains,
split into half-heads with the PSUM->SBUF copies alternating between the
scalar and vector engines so consecutive chunks overlap. PSUM->SBUF copies
are batched 4-wide (one 2 KiB bank, amortizes ~150ns/instruction).

Per-core layout trick: a head's [2048, 64] tensor is viewed as [128, 16, 64]
(partition p holds rows 16p..16p+15, 4 KiB contiguous DRAM per partition, so
every DMA is fully coalesced). The KV reduction over L is order-independent,
and the same interleaved row mapping flows through transpose -> matmul ->
store unchanged.
"""

import sys

if "/opt/trn_rl_repo" not in sys.path:
    sys.path.insert(0, "/opt/trn_rl_repo")

from contextlib import ExitStack

import numpy as np

import concourse.bass as bass
import concourse.tile as tile
from concourse import bacc, mybir
from concourse.bass_utils import run_bass_kernel_spmd

B, H, L, D = 2, 16, 2048, 64
N_CORES = 8
HPC = (B * H) // N_CORES  # heads per core = 4
P = 128
J = L // P  # 16 row-slots per partition
F32 = mybir.dt.float32
BF16 = mybir.dt.bfloat16


def _body(ctx: ExitStack, tc: tile.TileContext, o_d, qkv_d):
    nc = tc.nc

    const_pool = ctx.enter_context(tc.tile_pool(name="const", bufs=1))
    in_pool = ctx.enter_context(tc.tile_pool(name="in", bufs=4))
    kvb_pool = ctx.enter_context(tc.tile_pool(name="kvb", bufs=4))
    qt_pool = ctx.enter_context(tc.tile_pool(name="qt", bufs=8))
    kv_pool = ctx.enter_context(tc.tile_pool(name="kv", bufs=4))
    out_pool = ctx.enter_context(tc.tile_pool(name="out", bufs=4))
    psum_kv = ctx.enter_context(tc.tile_pool(name="psum_kv", bufs=1, space="PSUM"))
    psum_s = ctx.enter_context(tc.tile_pool(name="psum_s", bufs=1, space="PSUM"))
    psum_t = ctx.enter_context(tc.tile_pool(name="psum_t", bufs=2, space="PSUM"))
    psum_o = ctx.enter_context(tc.tile_pool(name="psum_o", bufs=2, space="PSUM"))
    psum_w = ctx.enter_context(tc.tile_pool(name="psum_w", bufs=1, space="PSUM"))

    qkv_sbs = [
        in_pool.tile([P, 3, J, D], F32, tag="qkv", name=f"qkv{h}") for h in range(HPC)
    ]
    kv_sbs = [
        kvb_pool.tile([P, 2, J, D], BF16, tag="kvb", name=f"kvb{h}")
        for h in range(HPC)
    ]

    def qkv_view(h):
        # [3, L, D] f32 in DRAM -> [p, t, j, d]; per partition chunks of
        # 4 KiB (q/k/v planes), fully coalesced descriptors.
        return qkv_d[h].rearrange("t (p j) d -> p t j d", p=P)

    # All loads issued up front on the sync queue (HWDGE); stores are emitted
    # later so their semaphore waits cannot delay a load. kv planes of heads
    # 2/3 arrive mid-stream (their chains finish under the stream); only the
    # q planes land late, and their post-load chain is short.
    nc.sync.dma_start(qkv_sbs[0][:], qkv_view(0))
    nc.sync.dma_start(qkv_sbs[1][:], qkv_view(1))
    nc.sync.dma_start(qkv_sbs[2][:, 1:3], qkv_view(2)[:, 1:3])  # kv2
    nc.sync.dma_start(qkv_sbs[3][:, 1:3], qkv_view(3)[:, 1:3])  # kv3
    nc.sync.dma_start(qkv_sbs[2][:, 0], qkv_view(2)[:, 0])  # q2
    nc.sync.dma_start(qkv_sbs[3][:, 0, 0:8], qkv_view(3)[:, 0, 0:8])  # q3a
    nc.sync.dma_start(qkv_sbs[3][:, 0, 8:J], qkv_view(3)[:, 0, 8:J])  # q3b

    # HAM warm-up: dense bf16 matmuls bridge from kernel start to the first
    # load's completion semaphore (~8 cold + 16 warm ~= 7us) so the PE runs
    # at 2.4 GHz when real work starts. Results are never read.
    warm_in = const_pool.tile([P, 4 * P], BF16)
    nc.vector.memset(warm_in[:], 0.0)
    warm_ps = psum_w.tile([P, 4 * P], F32)

    def warm_bundle(n):
        for _ in range(n):
            nc.tensor.matmul(
                warm_ps[:], warm_in[:, 0:P], warm_in[:], start=True, stop=True
            )

    warm_bundle(24)

    # Identity (f32, matching q's dtype) for PE transposes.
    ident = const_pool.tile([P, P], F32)
    nc.gpsimd.memset(ident[:], 0.0)
    nc.gpsimd.affine_select(
        out=ident[:],
        in_=ident[:],
        compare_op=mybir.AluOpType.not_equal,
        fill=1.0,
        base=0,
        pattern=[[-1, P]],
        channel_multiplier=1,
    )

    # ones_dbl[p, m] = 1 iff p == m (mod 64): one matmul against it both sums
    # the two column-tiled KV halves and replicates the result to partitions
    # 64..127 (the odd-slot block of KV2).
    ones_dbl = const_pool.tile([P, P], BF16)
    nc.gpsimd.memset(ones_dbl[:], 0.0)
    for off in (-64, 0, 64):
        nc.gpsimd.affine_select(
            out=ones_dbl[:],
            in_=ones_dbl[:],
            compare_op=mybir.AluOpType.not_equal,
            fill=1.0,
            base=-off,
            pattern=[[-1, P]],
            channel_multiplier=1,
        )

    qts_all = [[None, None] for _ in range(HPC)]
    kv2s = [None] * HPC

    def emit_T_group(h, g, dve=False):
        """Transpose q_h slab-pairs 4g..4g+3 into one PSUM bank, then one
        batched copy (f32 PSUM -> bf16 SBUF) on ACT (or DVE)."""
        q_sb = qkv_sbs[h][:, 0]
        qt_ps = psum_t.tile([P, 4, P], F32, tag="qt_ps")
        for i in range(4):
            jp = 4 * g + i
            nc.tensor.matmul(
                qt_ps[:, i],
                q_sb[:, 2 * jp : 2 * jp + 2],
                ident[:],
                is_transpose=True,
                start=True,
                stop=True,
                skip_group_check=True,
            )
        qt_sb = qt_pool.tile([P, 4, P], BF16, tag="qt", name=f"qt{h}_{g}")
        if dve:
            nc.vector.tensor_copy(qt_sb[:], qt_ps[:])
        else:
            nc.scalar.activation(
                qt_sb[:], qt_ps[:], mybir.ActivationFunctionType.Identity
            )
        qts_all[h][g] = qt_sb

    def emit_kv_chain(h):
        """Cast k/v_h -> bf16, KV accumulation, KV2 = blockdiag(KV, KV)."""
        k_sb = kv_sbs[h][:, 0]
        v_sb = kv_sbs[h][:, 1]

        nc.vector.tensor_copy(kv_sbs[h][:], qkv_sbs[h][:, 1:3])

        # KV = k.T @ v, column-tiled: even j-slots accumulate into PE columns
        # 0..63, odd slots into 64..127, so pair matmuls run concurrently.
        kv_ps = psum_kv.tile([P, D], F32)
        for jp in range(J // 2):
            nc.tensor.matmul(
                kv_ps[0:D],
                k_sb[:, 2 * jp],
                v_sb[:, 2 * jp],
                start=(jp == 0),
                stop=(jp == J // 2 - 1),
                tile_position=(0, 0),
                skip_group_check=True,
            )
            nc.tensor.matmul(
                kv_ps[D : 2 * D],
                k_sb[:, 2 * jp + 1],
                v_sb[:, 2 * jp + 1],
                start=(jp == 0),
                stop=(jp == J // 2 - 1),
                tile_position=(0, D),
                skip_group_check=True,
            )
        kv_raw = kv_pool.tile([P, D], BF16, tag="kv_raw", name=f"kvr{h}")
        nc.vector.tensor_copy(kv_raw[:], kv_ps[:])
        kv_st_ps = psum_s.tile([P, D], F32, tag="kv_st", name=f"kvs{h}")
        nc.tensor.matmul(kv_st_ps[:], ones_dbl[:], kv_raw[:], start=True, stop=True)
        kv2 = kv_pool.tile([P, P], BF16, tag="kv2", name=f"kv2_{h}")
        nc.gpsimd.memset(kv2[:], 0.0)
        nc.vector.tensor_copy(kv2[0:D, 0:D], kv_st_ps[0:D])
        nc.vector.tensor_copy(kv2[D : 2 * D, D : 2 * D], kv_st_ps[D : 2 * D])
        kv2s[h] = kv2

    out_sbs = [
        out_pool.tile([P, J, D], F32, tag="o", name=f"o{h}") for h in range(HPC)
    ]

    def emit_O_group(h, g, dve=False):
        """Out matmuls for slab-pairs 4g..4g+3, batched copy, then store."""
        out_sb = out_sbs[h]
        ov = o_d[h].rearrange("(p j) d -> p j d", p=P)
        o_ps = psum_o.tile([P, 8, D], F32, tag="o_ps")
        for i in range(4):
            nc.tensor.matmul(
                o_ps[:, 2 * i : 2 * i + 2],
                qts_all[h][g][:, i],
                kv2s[h][:],
                start=True,
                stop=True,
                skip_group_check=True,
            )
        half = slice(8 * g, 8 * g + 8)
        if dve:
            nc.vector.tensor_copy(out_sb[:, half], o_ps[:])
        else:
            nc.scalar.activation(
                out_sb[:, half], o_ps[:], mybir.ActivationFunctionType.Identity
            )
        nc.sync.dma_start(ov[:, half], out_sb[:, half])

    # Heads 0/1: full chains as their fused loads land (transposes before the
    # KV matmuls so the PE works during the DVE cast). One bundle fills the
    # data gap between O0 and head 2's chain.
    emit_T_group(0, 0)
    emit_T_group(0, 1)
    emit_kv_chain(0)
    emit_T_group(1, 0)
    emit_T_group(1, 1)
    emit_kv_chain(1)
    emit_O_group(0, 0)
    emit_O_group(0, 1)
    warm_bundle(12)
    emit_O_group(1, 0)
    emit_O_group(1, 1)
    # Heads 2/3: kv chains complete under the load stream.
    emit_kv_chain(2)
    emit_kv_chain(3)
    # q-side tails, half-head granularity, ACT/DVE alternating.
    emit_T_group(2, 0, dve=False)
    emit_O_group(2, 0, dve=False)
    emit_T_group(2, 1, dve=True)
    emit_O_group(2, 1, dve=True)
    emit_T_group(3, 0, dve=False)
    emit_O_group(3, 0, dve=False)
    emit_T_group(3, 1, dve=True)
    emit_O_group(3, 1, dve=True)


def build():
    nc = bacc.Bacc("TRN2", target_bir_lowering=False, debug=False)
    qkv_d = nc.dram_tensor("qkv", [HPC, 3, L, D], F32, kind="ExternalInput").ap()
    o_d = nc.dram_tensor("out", [HPC, L, D], F32, kind="ExternalOutput").ap()
    with tile.TileContext(nc) as tc, ExitStack() as ctx:
        _body(ctx, tc, o_d, qkv_d)
    nc.compile()
    return nc


_NC = None


def _get_nc():
    global _NC
    if _NC is None:
        _NC = build()
    return _NC


def make_in_maps(q, k, v):
    qf = np.asarray(q, dtype=np.float32).reshape(B * H, L, D)
    kf = np.asarray(k, dtype=np.float32).reshape(B * H, L, D)
    vf = np.asarray(v, dtype=np.float32).reshape(B * H, L, D)
    # [B*H, 3, L, D]: per head q/k/v adjacent so one DMA loads a whole head.
    qkv = np.stack([qf, kf, vf], axis=1)
    return [
        {"qkv": np.ascontiguousarray(qkv[c * HPC : (c + 1) * HPC])}
        for c in range(N_CORES)
    ]


def run_sharded(q, k, v, **spmd_kwargs):
    """Run on all 8 cores; returns (full_output, BassKernelResults)."""
    nc = _get_nc()
    res = run_bass_kernel_spmd(
        nc, make_in_maps(q, k, v), core_ids=list(range(N_CORES)), **spmd_kwargs
    )
    shards = [np.asarray(res.results[c]["out"]) for c in range(N_CORES)]
    out = np.concatenate(shards, axis=0).reshape(B, H, L, D).astype(np.float32)
    return out, res


def kernel(q, k, v):
    out, _ = run_sharded(q, k, v)
    return out
